# revision 12
# baseline (speedup 1.0000x reference)
"""Butterfly sparse-attention MLP kernel for 8 Trainium2 NeuronCores.

Computation (from the reference):
    attn = (w1.T @ w2.T) * sparse_mask          # [4096 s, 4096 t]
    y    = gelu(x @ attn + b2)                  # [8, 768, 4096]

sparse_mask is banded: mask[s, t] == 0 whenever |s - t| > 133.  Each core
owns a 512-wide t-block and only needs an 896-wide s-window around it.
Per t-subtile of 128, only 4 of the 7 s-chunks in the window can carry
non-zero attn, so phase B contracts over 512 of s instead of 4096.

Sharding: tensor-parallel over t (8 blocks of 512).  All per-core variation
is in the input data (windows are zero-padded at the edges; mask zeros make
padded contributions exactly zero), so one SPMD BIR serves all 8 cores.

DMA streams are spread across the sync/scalar/vector HW-DGE queues — a
single queue saturates at ~240 GB/s, below the ~360 GB/s HBM per core.
"""

import numpy as np

B, T, D = 8, 768, 4096
N = B * T            # 6144 rows of x
NCORES = 8
TB = 512             # t-columns per core
P = 128
MARGIN = 192         # s-window extends this far before/after the t-block
SW = TB + 2 * MARGIN  # 896 s-window width
NCH = SW // P        # 7 s-chunks
DCH = D // P         # 32 d-chunks (contraction of phase A)
NQ = TB // P         # 4 t-subtiles per core
GN = 1024            # n-group width in phase B
NG = N // GN         # 6 n-groups
MMN = 512            # moving-operand max for fp32 matmul
BANDCH = 4           # s-chunks feeding one t-subtile (covers +-133 band)

_NC = None


def _build_module():
    from concourse import bacc, bass, mybir, tile

    f32 = mybir.dt.float32
    f32r = mybir.dt.float32r
    PSUM = bass.MemorySpace.PSUM

    nc = bacc.Bacc("TRN2", target_bir_lowering=False, debug=False)
    xT_d = nc.declare_dram_parameter("xT_s", [SW, N], f32r, isOutput=False)
    w1_d = nc.declare_dram_parameter("w1_s", [D, SW], f32r, isOutput=False)
    w2T_d = nc.declare_dram_parameter("w2T_s", [D, TB], f32r, isOutput=False)
    mask_d = nc.declare_dram_parameter("mask_s", [SW, TB], f32, isOutput=False)
    b2_d = nc.declare_dram_parameter("b2c_s", [P, NQ], f32, isOutput=False)
    yT_d = nc.declare_dram_parameter("yT_s", [TB, N], f32, isOutput=True)

    with tile.TileContext(nc) as tc:
        with (
            tc.tile_pool(name="const", bufs=1) as cpool,
            tc.tile_pool(name="attn", bufs=1) as apool,
            tc.tile_pool(name="xp", bufs=4 * NCH) as xp,
            tc.tile_pool(name="yp", bufs=3) as yp,
        ):
            b2_t = cpool.tile([P, NQ], f32)
            nc.sync.dma_start(b2_t[:], b2_d[:])

            # ---- Phase A: attn[s, t] = (w1.T @ w2T) * mask on the band ----
            attn_sb = []
            with (
                tc.tile_pool(name="w1p", bufs=6) as w1p,
                tc.tile_pool(name="w2p", bufs=6) as w2p,
                tc.tile_pool(name="mp", bufs=2) as mp,
                tc.tile_pool(name="psA", bufs=1, space=PSUM) as psA,
            ):
                attn_ps = [
                    psA.tile([P, TB], f32, name=f"attn_ps{j}") for j in range(NCH)
                ]
                for k in range(DCH):
                    w1_t = w1p.tile([P, SW], f32r)
                    nc.sync.dma_start(w1_t[:], w1_d[k * P:(k + 1) * P, :])
                    w2_t = w2p.tile([P, TB], f32r)
                    nc.scalar.dma_start(w2_t[:], w2T_d[k * P:(k + 1) * P, :])
                    for j in range(NCH):
                        nc.tensor.matmul(
                            attn_ps[j][:],
                            w1_t[:, j * P:(j + 1) * P],
                            w2_t[:],
                            start=(k == 0),
                            stop=(k == DCH - 1),
                        )
                for j in range(NCH):
                    m_t = mp.tile([P, TB], f32)
                    nc.gpsimd.dma_start(m_t[:], mask_d[j * P:(j + 1) * P, :])
                    a_t = apool.tile([P, TB], f32r, name=f"attn_sb{j}")
                    nc.vector.tensor_mul(a_t[:], attn_ps[j][:], m_t[:])
                    attn_sb.append(a_t)

            # ---- Phase B: yT[t, n] = gelu(attn.T @ xT + b2) on the band ----
            with (
                tc.tile_pool(name="psB", bufs=3, space=PSUM) as psB,
            ):
                for g in range(NG):
                    x_t = []
                    for j in range(NCH):
                        xt = xp.tile([P, GN], f32r, name="x_t", tag="x_t")
                        nc.gpsimd.dma_start(
                            xt[:], xT_d[j * P:(j + 1) * P, g * GN:(g + 1) * GN]
                        )
                        x_t.append(xt)
                    for q in range(NQ):
                        y_ps = psB.tile([P, GN], f32)
                        for h in range(GN // MMN):
                            nsl = slice(h * MMN, (h + 1) * MMN)
                            for c in range(BANDCH):
                                j = q + c
                                nc.tensor.matmul(
                                    y_ps[:, nsl],
                                    attn_sb[j][:, q * P:(q + 1) * P],
                                    x_t[j][:, nsl],
                                    start=(c == 0),
                                    stop=(c == BANDCH - 1),
                                )
                        y_sb = yp.tile([P, GN], f32)
                        nc.scalar.activation(
                            y_sb[:],
                            y_ps[:],
                            mybir.ActivationFunctionType.Gelu,
                            bias=b2_t[:, q:q + 1],
                            scale=1.0,
                        )
                        st_eng = nc.sync if q % 2 == 0 else nc.scalar
                        st_eng.dma_start(
                            yT_d[q * P:(q + 1) * P, g * GN:(g + 1) * GN], y_sb[:]
                        )

    nc.compile()
    nc.finalize()
    return nc


def _get_nc():
    global _NC
    if _NC is None:
        _NC = _build_module()
    return _NC


def prepare_in_maps(x, w1, w2, b2, sparse_mask):
    x = np.asarray(x, dtype=np.float32)
    w1 = np.asarray(w1, dtype=np.float32)
    w2 = np.asarray(w2, dtype=np.float32)
    b2 = np.asarray(b2, dtype=np.float32)
    sparse_mask = np.asarray(sparse_mask, dtype=np.float32)

    xT = np.ascontiguousarray(x.reshape(N, D).T)          # [s, n]
    w2T = np.ascontiguousarray(w2.T)                      # [d, t]

    # Zero-pad the s axis by MARGIN on both sides so every core's window is
    # a plain slice; mask zeros make the padded rows contribute nothing.
    xT_pad = np.zeros((D + 2 * MARGIN, N), dtype=np.float32)
    xT_pad[MARGIN:MARGIN + D] = xT
    w1_pad = np.zeros((D, D + 2 * MARGIN), dtype=np.float32)
    w1_pad[:, MARGIN:MARGIN + D] = w1
    mask_pad = np.zeros((D + 2 * MARGIN, D), dtype=np.float32)
    mask_pad[MARGIN:MARGIN + D] = sparse_mask

    in_maps = []
    for i in range(NCORES):
        s0 = i * TB           # window start in padded coords = i*TB - 192 + 192
        t0 = i * TB
        in_maps.append({
            "xT_s": np.ascontiguousarray(xT_pad[s0:s0 + SW]),
            "w1_s": np.ascontiguousarray(w1_pad[:, s0:s0 + SW]),
            "w2T_s": np.ascontiguousarray(w2T[:, t0:t0 + TB]),
            "mask_s": np.ascontiguousarray(mask_pad[s0:s0 + SW, t0:t0 + TB]),
            "b2c_s": np.ascontiguousarray(
                b2[t0:t0 + TB].reshape(NQ, P).T
            ),
        })
    return in_maps


def assemble(results):
    out = np.empty((N, D), dtype=np.float32)
    for i in range(NCORES):
        out[:, i * TB:(i + 1) * TB] = results[i]["yT_s"].T
    return out.reshape(B, T, D)


def kernel(x, w1, w2, b2, sparse_mask):
    from concourse.bass_utils import run_bass_kernel_spmd

    in_maps = prepare_in_maps(x, w1, w2, b2, sparse_mask)
    nc = _get_nc()
    res = run_bass_kernel_spmd(nc, in_maps, list(range(NCORES)))
    return assemble(res.results)


# revision 14
# speedup vs baseline: 1.5333x; 1.5333x over previous
"""Butterfly sparse-attention MLP kernel for 8 Trainium2 NeuronCores.

Computation (from the reference):
    attn = (w1.T @ w2.T) * sparse_mask          # [4096 s, 4096 t]
    y    = gelu(x @ attn + b2)                  # [8, 768, 4096]

sparse_mask is banded: mask[s, t] == 0 whenever |s - t| > 133.  Each core
owns a 512-wide t-block and only needs an 896-wide s-window around it.
Per t-subtile of 128, only 4 of the 7 s-chunks in the window can carry
non-zero attn, so phase B contracts over 512 of s instead of 4096.

Sharding: tensor-parallel over t (8 blocks of 512).  All per-core variation
is in the input data (windows are zero-padded at the edges; mask zeros make
padded contributions exactly zero), so one SPMD BIR serves all 8 cores.

DMA streams are spread across the sync/scalar/vector HW-DGE queues — a
single queue saturates at ~240 GB/s, below the ~360 GB/s HBM per core.
"""

import numpy as np

B, T, D = 8, 768, 4096
N = B * T            # 6144 rows of x
NCORES = 8
TB = 512             # t-columns per core
P = 128
MARGIN = 192         # s-window extends this far before/after the t-block
SW = TB + 2 * MARGIN  # 896 s-window width
NCH = SW // P        # 7 s-chunks
DCH = D // P         # 32 d-chunks (contraction of phase A)
NQ = TB // P         # 4 t-subtiles per core
GN = 1024            # n-group width in phase B
NG = N // GN         # 6 n-groups
MMN = 512            # moving-operand max for fp32 matmul
BANDCH = 4           # s-chunks feeding one t-subtile (covers +-133 band)

_NC = None


def _build_module():
    from concourse import bacc, bass, mybir, tile

    f32 = mybir.dt.float32
    f16 = mybir.dt.float16
    PSUM = bass.MemorySpace.PSUM

    nc = bacc.Bacc("TRN2", target_bir_lowering=False, debug=False)
    xT_d = nc.declare_dram_parameter("xT_s", [SW, N], f16, isOutput=False)
    w1_d = nc.declare_dram_parameter("w1_s", [D, SW], f16, isOutput=False)
    w2T_d = nc.declare_dram_parameter("w2T_s", [D, TB], f16, isOutput=False)
    mask_d = nc.declare_dram_parameter("mask_s", [SW, TB], f32, isOutput=False)
    b2_d = nc.declare_dram_parameter("b2c_s", [P, NQ], f32, isOutput=False)
    yT_d = nc.declare_dram_parameter("yT_s", [TB, N], f32, isOutput=True)

    with tile.TileContext(nc) as tc:
        with (
            tc.tile_pool(name="const", bufs=1) as cpool,
            tc.tile_pool(name="attn", bufs=1) as apool,
            tc.tile_pool(name="xp", bufs=4 * NCH) as xp,
            tc.tile_pool(name="yp", bufs=3) as yp,
        ):
            b2_t = cpool.tile([P, NQ], f32)
            nc.sync.dma_start(b2_t[:], b2_d[:])

            # ---- Phase A: attn[s, t] = (w1.T @ w2T) * mask on the band ----
            attn_sb = []
            with (
                tc.tile_pool(name="w1p", bufs=6) as w1p,
                tc.tile_pool(name="w2p", bufs=6) as w2p,
                tc.tile_pool(name="mp", bufs=1) as mp,
                tc.tile_pool(name="psA", bufs=1, space=PSUM) as psA,
            ):
                attn_ps = [
                    psA.tile([P, TB], f32, name=f"attn_ps{j}") for j in range(NCH)
                ]
                m_ts = []
                for j in range(NCH):
                    m_t = mp.tile([P, TB], f32, name=f"m_t{j}")
                    nc.gpsimd.dma_start(m_t[:], mask_d[j * P:(j + 1) * P, :])
                    m_ts.append(m_t)
                for k in range(DCH):
                    w1_t = w1p.tile([P, SW], f16)
                    nc.sync.dma_start(w1_t[:], w1_d[k * P:(k + 1) * P, :])
                    w2_t = w2p.tile([P, TB], f16)
                    nc.scalar.dma_start(w2_t[:], w2T_d[k * P:(k + 1) * P, :])
                    for j in range(NCH):
                        nc.tensor.matmul(
                            attn_ps[j][:],
                            w1_t[:, j * P:(j + 1) * P],
                            w2_t[:],
                            start=(k == 0),
                            stop=(k == DCH - 1),
                        )
                for j in range(NCH):
                    a_t = apool.tile([P, TB], f16, name=f"attn_sb{j}")
                    nc.vector.tensor_mul(a_t[:], attn_ps[j][:], m_ts[j][:])
                    attn_sb.append(a_t)

            # ---- Phase B: yT[t, n] = gelu(attn.T @ xT + b2) on the band ----
            with (
                tc.tile_pool(name="psB", bufs=3, space=PSUM) as psB,
            ):
                for g in range(NG):
                    x_t = []
                    for j in range(NCH):
                        xt = xp.tile([P, GN], f16, name="x_t", tag="x_t")
                        nc.gpsimd.dma_start(
                            xt[:], xT_d[j * P:(j + 1) * P, g * GN:(g + 1) * GN]
                        )
                        x_t.append(xt)
                    for q in range(NQ):
                        y_ps = psB.tile([P, GN], f32)
                        for h in range(GN // MMN):
                            nsl = slice(h * MMN, (h + 1) * MMN)
                            for c in range(BANDCH):
                                j = q + c
                                nc.tensor.matmul(
                                    y_ps[:, nsl],
                                    attn_sb[j][:, q * P:(q + 1) * P],
                                    x_t[j][:, nsl],
                                    start=(c == 0),
                                    stop=(c == BANDCH - 1),
                                )
                        y_sb = yp.tile([P, GN], f32)
                        nc.scalar.activation(
                            y_sb[:],
                            y_ps[:],
                            mybir.ActivationFunctionType.Gelu,
                            bias=b2_t[:, q:q + 1],
                            scale=1.0,
                        )
                        st_eng = nc.sync if q % 2 == 0 else nc.scalar
                        st_eng.dma_start(
                            yT_d[q * P:(q + 1) * P, g * GN:(g + 1) * GN], y_sb[:]
                        )

    nc.compile()
    nc.finalize()
    return nc


def _get_nc():
    global _NC
    if _NC is None:
        _NC = _build_module()
    return _NC


def prepare_in_maps(x, w1, w2, b2, sparse_mask):
    x = np.asarray(x, dtype=np.float32)
    w1 = np.asarray(w1, dtype=np.float32)
    w2 = np.asarray(w2, dtype=np.float32)
    b2 = np.asarray(b2, dtype=np.float32)
    sparse_mask = np.asarray(sparse_mask, dtype=np.float32)

    xT = np.ascontiguousarray(x.reshape(N, D).T.astype(np.float16))   # [s, n]
    w2T = np.ascontiguousarray(w2.T.astype(np.float16))               # [d, t]

    # Zero-pad the s axis by MARGIN on both sides so every core's window is
    # a plain slice; mask zeros make the padded rows contribute nothing.
    xT_pad = np.zeros((D + 2 * MARGIN, N), dtype=np.float16)
    xT_pad[MARGIN:MARGIN + D] = xT
    w1_pad = np.zeros((D, D + 2 * MARGIN), dtype=np.float16)
    w1_pad[:, MARGIN:MARGIN + D] = w1.astype(np.float16)
    mask_pad = np.zeros((D + 2 * MARGIN, D), dtype=np.float32)
    mask_pad[MARGIN:MARGIN + D] = sparse_mask

    in_maps = []
    for i in range(NCORES):
        s0 = i * TB           # window start in padded coords = i*TB - 192 + 192
        t0 = i * TB
        in_maps.append({
            "xT_s": np.ascontiguousarray(xT_pad[s0:s0 + SW]),
            "w1_s": np.ascontiguousarray(w1_pad[:, s0:s0 + SW]),
            "w2T_s": np.ascontiguousarray(w2T[:, t0:t0 + TB]),
            "mask_s": np.ascontiguousarray(mask_pad[s0:s0 + SW, t0:t0 + TB]),
            "b2c_s": np.ascontiguousarray(
                b2[t0:t0 + TB].reshape(NQ, P).T
            ),
        })
    return in_maps


def assemble(results):
    out = np.empty((N, D), dtype=np.float32)
    for i in range(NCORES):
        out[:, i * TB:(i + 1) * TB] = results[i]["yT_s"].T
    return out.reshape(B, T, D)


def kernel(x, w1, w2, b2, sparse_mask):
    from concourse.bass_utils import run_bass_kernel_spmd

    in_maps = prepare_in_maps(x, w1, w2, b2, sparse_mask)
    nc = _get_nc()
    res = run_bass_kernel_spmd(nc, in_maps, list(range(NCORES)))
    return assemble(res.results)


# revision 15
# speedup vs baseline: 1.6197x; 1.0564x over previous
"""Butterfly sparse-attention MLP kernel for 8 Trainium2 NeuronCores.

Computation (from the reference):
    attn = (w1.T @ w2.T) * sparse_mask          # [4096 s, 4096 t]
    y    = gelu(x @ attn + b2)                  # [8, 768, 4096]

sparse_mask is banded: mask[s, t] == 0 whenever |s - t| > 133.  Each core
owns a 512-wide t-block and only needs an 896-wide s-window around it.
Per t-subtile of 128, only 4 of the 7 s-chunks in the window can carry
non-zero attn, so phase B contracts over 512 of s instead of 4096.

Sharding: tensor-parallel over t (8 blocks of 512).  All per-core variation
is in the input data (windows are zero-padded at the edges; mask zeros make
padded contributions exactly zero), so one SPMD BIR serves all 8 cores.

DMA streams are spread across the sync/scalar/vector HW-DGE queues — a
single queue saturates at ~240 GB/s, below the ~360 GB/s HBM per core.
"""

import numpy as np

B, T, D = 8, 768, 4096
N = B * T            # 6144 rows of x
NCORES = 8
TB = 512             # t-columns per core
P = 128
MARGIN = 192         # s-window extends this far before/after the t-block
SW = TB + 2 * MARGIN  # 896 s-window width
NCH = SW // P        # 7 s-chunks
DCH = D // P         # 32 d-chunks (contraction of phase A)
NQ = TB // P         # 4 t-subtiles per core
GN = 1024            # n-group width in phase B
NG = N // GN         # 6 n-groups
MMN = 512            # moving-operand max for fp32 matmul
BANDCH = 4           # s-chunks feeding one t-subtile (covers +-133 band)

_NC = None


def _build_module():
    from concourse import bacc, bass, mybir, tile

    f32 = mybir.dt.float32
    f16 = mybir.dt.float16
    PSUM = bass.MemorySpace.PSUM

    nc = bacc.Bacc("TRN2", target_bir_lowering=False, debug=False)
    xT_d = nc.declare_dram_parameter("xT_s", [SW, N], f16, isOutput=False)
    w1_d = nc.declare_dram_parameter("w1_s", [D, SW], f16, isOutput=False)
    w2T_d = nc.declare_dram_parameter("w2T_s", [D, TB], f16, isOutput=False)
    mask_d = nc.declare_dram_parameter("mask_s", [SW, TB], f32, isOutput=False)
    b2_d = nc.declare_dram_parameter("b2c_s", [P, NQ], f32, isOutput=False)
    yT_d = nc.declare_dram_parameter("yT_s", [TB, N], f32, isOutput=True)

    with tile.TileContext(nc) as tc:
        with (
            tc.tile_pool(name="const", bufs=1) as cpool,
            tc.tile_pool(name="attn", bufs=1) as apool,
            tc.tile_pool(name="xp", bufs=NG * NCH) as xp,
            tc.tile_pool(name="yp", bufs=3) as yp,
        ):
            b2_t = cpool.tile([P, NQ], f32)
            nc.sync.dma_start(b2_t[:], b2_d[:])

            # ---- Phase A: attn[s, t] = (w1.T @ w2T) * mask on the band ----
            attn_sb = []
            with (
                tc.tile_pool(name="w1p", bufs=6) as w1p,
                tc.tile_pool(name="w2p", bufs=6) as w2p,
                tc.tile_pool(name="mp", bufs=1) as mp,
                tc.tile_pool(name="psA", bufs=1, space=PSUM) as psA,
            ):
                attn_ps = [
                    psA.tile([P, TB], f32, name=f"attn_ps{j}") for j in range(NCH)
                ]
                m_ts = []
                for j in range(NCH):
                    m_t = mp.tile([P, TB], f32, name=f"m_t{j}")
                    nc.gpsimd.dma_start(m_t[:], mask_d[j * P:(j + 1) * P, :])
                    m_ts.append(m_t)
                for k in range(DCH):
                    e1, e2 = (nc.sync, nc.scalar) if k % 2 == 0 else (nc.scalar, nc.sync)
                    w1_t = w1p.tile([P, SW], f16)
                    e1.dma_start(w1_t[:], w1_d[k * P:(k + 1) * P, :])
                    w2_t = w2p.tile([P, TB], f16)
                    e2.dma_start(w2_t[:], w2T_d[k * P:(k + 1) * P, :])
                    for j in range(NCH):
                        lo = P * max(0, j - (BANDCH - 1))
                        hi = P * min(NQ - 1, j) + P
                        nc.tensor.matmul(
                            attn_ps[j][:, lo:hi],
                            w1_t[:, j * P:(j + 1) * P],
                            w2_t[:, lo:hi],
                            start=(k == 0),
                            stop=(k == DCH - 1),
                        )
                for j in range(NCH):
                    lo = P * max(0, j - (BANDCH - 1))
                    hi = P * min(NQ - 1, j) + P
                    a_t = apool.tile([P, TB], f16, name=f"attn_sb{j}")
                    nc.vector.tensor_mul(
                        a_t[:, lo:hi], attn_ps[j][:, lo:hi], m_ts[j][:, lo:hi]
                    )
                    attn_sb.append(a_t)

            # ---- Phase B: yT[t, n] = gelu(attn.T @ xT + b2) on the band ----
            with (
                tc.tile_pool(name="psB", bufs=3, space=PSUM) as psB,
            ):
                for g in range(NG):
                    x_t = []
                    for j in range(NCH):
                        xt = xp.tile([P, GN], f16, name="x_t", tag="x_t")
                        nc.gpsimd.dma_start(
                            xt[:], xT_d[j * P:(j + 1) * P, g * GN:(g + 1) * GN]
                        )
                        x_t.append(xt)
                    for q in range(NQ):
                        y_ps = psB.tile([P, GN], f32)
                        for h in range(GN // MMN):
                            nsl = slice(h * MMN, (h + 1) * MMN)
                            for c in range(BANDCH):
                                j = q + c
                                nc.tensor.matmul(
                                    y_ps[:, nsl],
                                    attn_sb[j][:, q * P:(q + 1) * P],
                                    x_t[j][:, nsl],
                                    start=(c == 0),
                                    stop=(c == BANDCH - 1),
                                )
                        y_sb = yp.tile([P, GN], f32)
                        nc.scalar.activation(
                            y_sb[:],
                            y_ps[:],
                            mybir.ActivationFunctionType.Gelu,
                            bias=b2_t[:, q:q + 1],
                            scale=1.0,
                        )
                        st_eng = nc.sync if q % 2 == 0 else nc.scalar
                        st_eng.dma_start(
                            yT_d[q * P:(q + 1) * P, g * GN:(g + 1) * GN], y_sb[:]
                        )

    nc.compile()
    nc.finalize()
    return nc


def _get_nc():
    global _NC
    if _NC is None:
        _NC = _build_module()
    return _NC


def prepare_in_maps(x, w1, w2, b2, sparse_mask):
    x = np.asarray(x, dtype=np.float32)
    w1 = np.asarray(w1, dtype=np.float32)
    w2 = np.asarray(w2, dtype=np.float32)
    b2 = np.asarray(b2, dtype=np.float32)
    sparse_mask = np.asarray(sparse_mask, dtype=np.float32)

    xT = np.ascontiguousarray(x.reshape(N, D).T.astype(np.float16))   # [s, n]
    w2T = np.ascontiguousarray(w2.T.astype(np.float16))               # [d, t]

    # Zero-pad the s axis by MARGIN on both sides so every core's window is
    # a plain slice; mask zeros make the padded rows contribute nothing.
    xT_pad = np.zeros((D + 2 * MARGIN, N), dtype=np.float16)
    xT_pad[MARGIN:MARGIN + D] = xT
    w1_pad = np.zeros((D, D + 2 * MARGIN), dtype=np.float16)
    w1_pad[:, MARGIN:MARGIN + D] = w1.astype(np.float16)
    mask_pad = np.zeros((D + 2 * MARGIN, D), dtype=np.float32)
    mask_pad[MARGIN:MARGIN + D] = sparse_mask

    in_maps = []
    for i in range(NCORES):
        s0 = i * TB           # window start in padded coords = i*TB - 192 + 192
        t0 = i * TB
        in_maps.append({
            "xT_s": np.ascontiguousarray(xT_pad[s0:s0 + SW]),
            "w1_s": np.ascontiguousarray(w1_pad[:, s0:s0 + SW]),
            "w2T_s": np.ascontiguousarray(w2T[:, t0:t0 + TB]),
            "mask_s": np.ascontiguousarray(mask_pad[s0:s0 + SW, t0:t0 + TB]),
            "b2c_s": np.ascontiguousarray(
                b2[t0:t0 + TB].reshape(NQ, P).T
            ),
        })
    return in_maps


def assemble(results):
    out = np.empty((N, D), dtype=np.float32)
    for i in range(NCORES):
        out[:, i * TB:(i + 1) * TB] = results[i]["yT_s"].T
    return out.reshape(B, T, D)


def kernel(x, w1, w2, b2, sparse_mask):
    from concourse.bass_utils import run_bass_kernel_spmd

    in_maps = prepare_in_maps(x, w1, w2, b2, sparse_mask)
    nc = _get_nc()
    res = run_bass_kernel_spmd(nc, in_maps, list(range(NCORES)))
    return assemble(res.results)


# revision 16
# speedup vs baseline: 1.8016x; 1.1123x over previous
"""Butterfly sparse-attention MLP kernel for 8 Trainium2 NeuronCores.

Computation (from the reference):
    attn = (w1.T @ w2.T) * sparse_mask          # [4096 s, 4096 t]
    y    = gelu(x @ attn + b2)                  # [8, 768, 4096]

sparse_mask is banded: mask[s, t] == 0 whenever |s - t| > 133.  Each core
owns a 512-wide t-block and only needs an 896-wide s-window around it.
Per t-subtile of 128, only 4 of the 7 s-chunks in the window can carry
non-zero attn, so phase B contracts over 512 of s instead of 4096.

Sharding: tensor-parallel over t (8 blocks of 512).  All per-core variation
is in the input data (windows are zero-padded at the edges; mask zeros make
padded contributions exactly zero), so one SPMD BIR serves all 8 cores.

DMA streams are spread across the sync/scalar/vector HW-DGE queues — a
single queue saturates at ~240 GB/s, below the ~360 GB/s HBM per core.
"""

import numpy as np

B, T, D = 8, 768, 4096
N = B * T            # 6144 rows of x
NCORES = 8
TB = 512             # t-columns per core
P = 128
MARGIN = 192         # s-window extends this far before/after the t-block
SW = TB + 2 * MARGIN  # 896 s-window width
NCH = SW // P        # 7 s-chunks
DCH = D // P         # 32 d-chunks (contraction of phase A)
NQ = TB // P         # 4 t-subtiles per core
GN = 1024            # n-group width in phase B
NG = N // GN         # 6 n-groups
MMN = 512            # moving-operand max for fp32 matmul
BANDCH = 4           # s-chunks feeding one t-subtile (covers +-133 band)

_NC = None


def _build_module():
    from concourse import bacc, bass, mybir, tile

    f32 = mybir.dt.float32
    f16 = mybir.dt.float16
    PSUM = bass.MemorySpace.PSUM

    nc = bacc.Bacc("TRN2", target_bir_lowering=False, debug=False)
    xT_d = nc.declare_dram_parameter("xT_s", [SW, N], f16, isOutput=False)
    w1_d = nc.declare_dram_parameter("w1_s", [D, SW], f16, isOutput=False)
    w2T_d = nc.declare_dram_parameter("w2T_s", [D, TB], f16, isOutput=False)
    mask_d = nc.declare_dram_parameter("mask_s", [SW, TB], f16, isOutput=False)
    b2_d = nc.declare_dram_parameter("b2c_s", [P, NQ], f32, isOutput=False)
    yT_d = nc.declare_dram_parameter("yT_s", [TB, N], f16, isOutput=True)

    with tile.TileContext(nc) as tc:
        with (
            tc.tile_pool(name="const", bufs=1) as cpool,
            tc.tile_pool(name="attn", bufs=1) as apool,
            tc.tile_pool(name="xp", bufs=2 * NCH) as xp,
            tc.tile_pool(name="yp", bufs=3) as yp,
        ):
            b2_t = cpool.tile([P, NQ], f32)
            nc.sync.dma_start(b2_t[:], b2_d[:])

            # ---- Phase A: attn[s, t] = (w1.T @ w2T) * mask on the band ----
            attn_sb = []
            with (
                tc.tile_pool(name="w1p", bufs=6) as w1p,
                tc.tile_pool(name="w2p", bufs=6) as w2p,
                tc.tile_pool(name="mp", bufs=1) as mp,
                tc.tile_pool(name="psA", bufs=1, space=PSUM) as psA,
            ):
                attn_ps = [
                    psA.tile([P, TB], f32, name=f"attn_ps{j}") for j in range(NCH)
                ]
                m_ts = []
                for j in range(NCH):
                    m_t = mp.tile([P, TB], f16, name=f"m_t{j}")
                    nc.gpsimd.dma_start(m_t[:], mask_d[j * P:(j + 1) * P, :])
                    m_ts.append(m_t)
                engs = [nc.sync, nc.scalar, nc.gpsimd]
                for k in range(DCH):
                    w1_t = w1p.tile([P, SW], f16)
                    engs[k % 3].dma_start(w1_t[:], w1_d[k * P:(k + 1) * P, :])
                    w2_t = w2p.tile([P, TB], f16)
                    engs[(k + 1) % 3].dma_start(w2_t[:], w2T_d[k * P:(k + 1) * P, :])
                    for j in range(NCH):
                        lo = P * max(0, j - (BANDCH - 1))
                        hi = P * min(NQ - 1, j) + P
                        nc.tensor.matmul(
                            attn_ps[j][:, lo:hi],
                            w1_t[:, j * P:(j + 1) * P],
                            w2_t[:, lo:hi],
                            start=(k == 0),
                            stop=(k == DCH - 1),
                        )
                for j in range(NCH):
                    lo = P * max(0, j - (BANDCH - 1))
                    hi = P * min(NQ - 1, j) + P
                    a_t = apool.tile([P, TB], f16, name=f"attn_sb{j}")
                    nc.vector.tensor_mul(
                        a_t[:, lo:hi], attn_ps[j][:, lo:hi], m_ts[j][:, lo:hi]
                    )
                    attn_sb.append(a_t)

            # ---- Phase B: yT[t, n] = gelu(attn.T @ xT + b2) on the band ----
            with (
                tc.tile_pool(name="psB", bufs=3, space=PSUM) as psB,
            ):
                for g in range(NG):
                    x_t = []
                    xengs = [nc.gpsimd, nc.sync, nc.scalar, nc.gpsimd,
                             nc.sync, nc.scalar, nc.gpsimd]
                    for j in range(NCH):
                        xt = xp.tile([P, GN], f16, name="x_t", tag="x_t")
                        xengs[j].dma_start(
                            xt[:], xT_d[j * P:(j + 1) * P, g * GN:(g + 1) * GN]
                        )
                        x_t.append(xt)
                    for q in range(NQ):
                        y_ps = psB.tile([P, GN], f32)
                        for h in range(GN // MMN):
                            nsl = slice(h * MMN, (h + 1) * MMN)
                            for c in range(BANDCH):
                                j = q + c
                                nc.tensor.matmul(
                                    y_ps[:, nsl],
                                    attn_sb[j][:, q * P:(q + 1) * P],
                                    x_t[j][:, nsl],
                                    start=(c == 0),
                                    stop=(c == BANDCH - 1),
                                )
                        y_sb = yp.tile([P, GN], f16)
                        nc.scalar.activation(
                            y_sb[:],
                            y_ps[:],
                            mybir.ActivationFunctionType.Gelu,
                            bias=b2_t[:, q:q + 1],
                            scale=1.0,
                        )
                        st_eng = nc.sync if q % 2 == 0 else nc.scalar
                        st_eng.dma_start(
                            yT_d[q * P:(q + 1) * P, g * GN:(g + 1) * GN], y_sb[:]
                        )

    nc.compile()
    nc.finalize()
    return nc


def _get_nc():
    global _NC
    if _NC is None:
        _NC = _build_module()
    return _NC


def prepare_in_maps(x, w1, w2, b2, sparse_mask):
    x = np.asarray(x, dtype=np.float32)
    w1 = np.asarray(w1, dtype=np.float32)
    w2 = np.asarray(w2, dtype=np.float32)
    b2 = np.asarray(b2, dtype=np.float32)
    sparse_mask = np.asarray(sparse_mask, dtype=np.float32)

    xT = np.ascontiguousarray(x.reshape(N, D).T.astype(np.float16))   # [s, n]
    w2T = np.ascontiguousarray(w2.T.astype(np.float16))               # [d, t]

    # Zero-pad the s axis by MARGIN on both sides so every core's window is
    # a plain slice; mask zeros make the padded rows contribute nothing.
    xT_pad = np.zeros((D + 2 * MARGIN, N), dtype=np.float16)
    xT_pad[MARGIN:MARGIN + D] = xT
    w1_pad = np.zeros((D, D + 2 * MARGIN), dtype=np.float16)
    w1_pad[:, MARGIN:MARGIN + D] = w1.astype(np.float16)
    mask_pad = np.zeros((D + 2 * MARGIN, D), dtype=np.float16)
    mask_pad[MARGIN:MARGIN + D] = sparse_mask.astype(np.float16)

    in_maps = []
    for i in range(NCORES):
        s0 = i * TB           # window start in padded coords = i*TB - 192 + 192
        t0 = i * TB
        in_maps.append({
            "xT_s": np.ascontiguousarray(xT_pad[s0:s0 + SW]),
            "w1_s": np.ascontiguousarray(w1_pad[:, s0:s0 + SW]),
            "w2T_s": np.ascontiguousarray(w2T[:, t0:t0 + TB]),
            "mask_s": np.ascontiguousarray(mask_pad[s0:s0 + SW, t0:t0 + TB]),
            "b2c_s": np.ascontiguousarray(
                b2[t0:t0 + TB].reshape(NQ, P).T
            ),
        })
    return in_maps


def assemble(results):
    out = np.empty((N, D), dtype=np.float32)
    for i in range(NCORES):
        out[:, i * TB:(i + 1) * TB] = results[i]["yT_s"].T.astype(np.float32)
    return out.reshape(B, T, D)


def kernel(x, w1, w2, b2, sparse_mask):
    from concourse.bass_utils import run_bass_kernel_spmd

    in_maps = prepare_in_maps(x, w1, w2, b2, sparse_mask)
    nc = _get_nc()
    res = run_bass_kernel_spmd(nc, in_maps, list(range(NCORES)))
    return assemble(res.results)
